# revision 1
# baseline (speedup 1.0000x reference)
"""Trainium2 Bass kernel for nn_Dictionnary (convolutional sparse coding /
FISTA dictionary inference), data-parallel over the batch axis: each of the
8 NeuronCores processes one batch image independently (4096 patches/core).

Math (per unroll, mirrors the jax reference exactly):
  q' = mu * Af @ im2col(goal)                      [128, 4096]
  FISTA, 15 iters + 1 extra prox step, reformulated so the momentum is
  folded into pre-scaled weight matrices (W symmetric):
      s_i  = (1+b)W d_i + (-b)W d_{i-1} + q'       (2 matmuls, PSUM accum)
      d_i+1 = prox(s_i) = relu(s_i-lam) - relu(-s_i-lam)
  pred^T = Af^T cf + patch_mean ; premultiplied by vinv fold windows
  goal   = y_sc + fold(pred^T)   via scatter-DMA + ones-matmul reduction

The prox(+q) is one fused custom DVE op; FISTA iterates and the small
matmul operands are bf16 (PSUM accumulation stays fp32).
Host side: atom normalization (needs an exact spectral norm), the scaled
weight stack, the unroll-0 q' (goal==y), and per-image constants.
"""
import numpy as np

N = 128          # atoms
A = 12           # atom size
A2 = 144         # atom pixels
B = 8            # batch
HW = 75
PH = 64          # patch grid
NP = PH * PH     # 4096 patches per core
PIX = HW * HW    # 5625
LAM = 0.1
UNROLL = 2
ITERS = 15
FC = 512         # FISTA free-dim chunk (one PSUM bank of fp32)
NCH = NP // FC   # 8 chunks
RC = 375         # reduce chunk = 5 rows of 75
NRC = PIX // RC  # 15 chunks

DEBUG = False
_PROX_OP = None


def _host_prep(atoms, beta, mu):
    beta = float(max(beta, 0.0))
    mu = float(max(mu, 0.0))
    Araw = atoms - atoms.mean(axis=(1, 2, 3), keepdims=True)
    Af = Araw.reshape(N, -1).astype(np.float64)
    Af = Af / np.linalg.norm(Af, axis=1, keepdims=True)
    Af = Af / (np.linalg.norm(Af, ord=2) * np.sqrt(mu))
    Af = Af.astype(np.float32)
    W = np.eye(N, dtype=np.float32) - np.float32(mu) * (Af @ Af.T)
    t = 1.0
    alphas = []
    for _ in range(ITERS):
        tn = (1.0 + np.sqrt(1.0 + 4.0 * t * t)) / 2.0
        alphas.append((t - 1.0) / tn)
        t = tn
    wstack = [W]
    for i in range(1, ITERS):
        b_ = np.float32(alphas[i - 1])
        wstack += [(1 + b_) * W, (-b_) * W]
    wstack = np.ascontiguousarray(np.stack(wstack))          # [29,128,128]
    div = np.zeros((HW, HW), np.float32)
    for di in range(A):
        for dj in range(A):
            div[di:di + PH, dj:dj + PH] += 1.0
    denom = 1.0 + beta * div
    vinv = (beta / denom).astype(np.float32)
    return Af, wstack, np.float32(mu), denom, vinv


def _im2col(img):
    out = np.empty((A2, NP), np.float32)
    for di in range(A):
        for dj in range(A):
            out[di * A + dj] = img[di:di + PH, dj:dj + PH].reshape(-1)
    return out


def _get_prox_op():
    """Register (once) a fused DVE op: out = prox(in0 + in1, lam=imm2)."""
    global _PROX_OP
    if _PROX_OP is not None:
        return _PROX_OP
    import concourse.dve_ops as dve_ops
    from concourse.dve_spec import Spec, Src0, Src1, Zero, C2, relu, lower

    def _ref(in0, in1, s0, s1, imm2):
        u = in0.astype(np.float32) + in1.astype(np.float32)
        return np.maximum(u - imm2, 0.0) - np.maximum(-u - imm2, 0.0)

    spec = Spec(
        body=relu((Src0 + Src1) - C2) - relu((Zero - (Src0 + Src1)) - C2),
        reference=_ref,
    )
    op = dve_ops.DveOp("PROX_ADD_ANT", spec, subdim=False, uops_sha={})
    dve_ops.OPS.append(op)
    dve_ops.CUSTOM_DVE_SPECS[op.name] = op.spec
    dve_ops._SUB_OPCODE_FOR_NAME[op.name] = (
        dve_ops._CUSTOM_DVE_ROW_BASE + len(dve_ops.OPS) - 1)
    # pin the uop shas (computed locally; validated against HW by test.py)
    from concourse.dve_ops import DveOpSpec, has_src1, get_dve_sub_opcode
    for ver in ("v3", "v4"):
        res = DveOpSpec(name=op.name, opcode=get_dve_sub_opcode(op.name),
                        uops=lower(op.spec, ver=ver), rd1_en=has_src1(op.spec))
        op.uops_sha[ver] = res.sha(ver)
    _PROX_OP = op
    return op


def _build_program():
    import concourse.bacc as bacc
    import concourse.bass as bass
    import concourse.mybir as mybir
    import concourse.tile as tile

    f32 = mybir.dt.float32
    bf16 = mybir.dt.bfloat16
    prox_op = _get_prox_op()

    nc = bacc.Bacc(None, target_bir_lowering=False, num_swdge_queues=4)

    d_wstack = nc.dram_tensor("wstack", [29, N, N], bf16, kind="ExternalInput")
    d_afq = nc.dram_tensor("afq", [A2, N], bf16, kind="ExternalInput")
    d_afp = nc.dram_tensor("afp", [N, A2], bf16, kind="ExternalInput")
    d_pm = nc.dram_tensor("pmv", [1, NP], bf16, kind="ExternalInput")
    d_vw = nc.dram_tensor("vw", [A2, NP], bf16, kind="ExternalInput")
    d_q0 = nc.dram_tensor("q0", [N, NP], bf16, kind="ExternalInput")
    d_ysc = nc.dram_tensor("ysc", [1, PIX], f32, kind="ExternalInput")
    d_stga = nc.dram_tensor("stga", [72, PIX], bf16)
    d_stgb = nc.dram_tensor("stgb", [72, PIX], bf16)
    d_pred = nc.dram_tensor("pred2", [A2, NP], bf16, kind="ExternalOutput")
    d_goal = nc.dram_tensor("goalimg", [1, PIX], bf16)

    with tile.TileContext(nc) as tc:
        with (
            tc.tile_pool(name="cst", bufs=1) as cst,
            tc.tile_pool(name="gst", bufs=2) as gst,
            tc.tile_pool(name="psA", bufs=3, space="PSUM") as psA,
            tc.tile_pool(name="psB", bufs=2, space="PSUM") as psB,
        ):
            # ---- persistent tiles ----
            w_s = cst.tile([N, 29 * N], bf16)         # weight stack
            afq128 = cst.tile([N, N], bf16)
            afq16 = cst.tile([16, N], bf16)
            afp = cst.tile([N, A2], bf16)
            ones1 = cst.tile([1, N], bf16)            # lhsT for patch-mean add
            on128 = cst.tile([N, 1], bf16)            # reduce lhsT
            on16 = cst.tile([16, 1], bf16)
            zeros = cst.tile([N, 2 * FC], bf16)       # for prox(0 + q)
            pm = cst.tile([1, NP], bf16)
            vw128 = cst.tile([N, NP], bf16)
            vw16 = cst.tile([16, NP], bf16)
            qt = cst.tile([N, NP], bf16)              # q' tile
            dA = cst.tile([N, NP], bf16)              # FISTA d parity buffers
            dB = cst.tile([N, NP], bf16)
            pp128 = cst.tile([N, NP], bf16)           # im2col patches / pred2
            pp16 = cst.tile([16, NP], bf16)
            ctb128 = cst.tile([N, PIX], bf16)         # fold accumulator rows
            ctb16 = cst.tile([16, PIX], bf16)
            ysc = cst.tile([1, PIX], f32)

            # ---- loads / init ----
            sy = nc.sync
            for wi in range(29):
                sy.dma_start(w_s[:, wi * N:(wi + 1) * N], d_wstack[wi])
            for c in range(4):
                sl = slice(c * NP // 4, (c + 1) * NP // 4)
                nc.scalar.dma_start(qt[:, sl], d_q0[:, sl])
            nc.gpsimd.memset(ones1[:], 1.0)
            nc.gpsimd.memset(on128[:], 1.0)
            nc.gpsimd.memset(on16[:], 1.0)
            nc.gpsimd.memset(zeros[:], 0.0)
            nc.gpsimd.memset(ctb128[:], 0.0)
            nc.gpsimd.memset(ctb16[:], 0.0)

            def wsl(i):  # weight i as lhsT [128,128]
                return w_s[:, i * N:(i + 1) * N]

            def prox(dst, ps_ap, q_ap):
                return nc.vector._custom_dve(prox_op, out=dst, in0=ps_ap,
                                             in1=q_ap, imm2=LAM)

            cur, prv = dA, dB
            pre_ps = []
            for u_ in range(UNROLL):
                if u_ == 1:
                    # im2col: fused per-di DMAs from the DRAM goal image
                    # (DRAM src: compound +1-elem stride per partition ok)
                    GRPS = [(0 + 12 * d, 12) for d in range(10)] + \
                           [(120, 8), (128, 4), (132, 12)]
                    for gi, (k0, cnt) in enumerate(GRPS):
                        di, dj0 = divmod(k0, A)
                        s_ap = bass.AP(d_goal[:].tensor, di * HW + dj0,
                                       [[1, cnt], [HW, PH], [1, PH]])
                        if k0 < N:
                            d_ap = bass.AP(pp128[:].tensor, k0 * NP,
                                           [[NP, cnt], [PH, PH], [1, PH]])
                        else:
                            d_ap = bass.AP(pp16[:].tensor, (k0 - N) * NP,
                                           [[NP, cnt], [PH, PH], [1, PH]])
                        eng = (sy, nc.scalar)[gi % 2]
                        eng.dma_start(d_ap, s_ap)
                    for c in range(NCH):
                        ps = psB.tile([N, FC], f32, tag="psr")
                        sl = slice(c * FC, (c + 1) * FC)
                        nc.tensor.matmul(ps[:], afq128[:], pp128[:, sl],
                                         start=True, stop=False)
                        nc.tensor.matmul(ps[:], afq16[:], pp16[:, sl],
                                         start=False, stop=True)
                        nc.scalar.copy(qt[:, sl], ps[:])

                # ---- FISTA: 15 iters + final differentiable prox ----
                FC2 = 2 * FC
                for i in range(ITERS + 1):
                    if u_ == 0 and i == 0:
                        for c in range(NCH // 2):
                            sl = slice(c * FC2, (c + 1) * FC2)
                            prox(prv[:, sl], zeros[:], qt[:, sl])
                    elif u_ == 1 and i == 0 and pre_ps:
                        for c in range(NCH // 2):
                            sl = slice(c * FC2, (c + 1) * FC2)
                            prox(prv[:, sl], pre_ps[c][:], qt[:, sl])
                        pre_ps = []
                    else:
                        pair = not (i == 0 or i == ITERS or (u_ == 0 and i == 1))
                        if i == 0 or i == ITERS:
                            w1 = wsl(0)
                        elif u_ == 0 and i == 1:
                            w1 = wsl(1)
                        else:
                            w1 = wsl(2 * i - 1)
                        pss = []
                        for c in range(NCH // 2):
                            ps = psA.tile([N, FC2], f32, tag="ps")
                            pss.append(ps)
                            for h in range(2):
                                sl = slice(c * FC2 + h * FC,
                                           c * FC2 + (h + 1) * FC)
                                nc.tensor.matmul(ps[:, h * FC:(h + 1) * FC],
                                                 w1, cur[:, sl],
                                                 start=True, stop=not pair)
                        if pair:
                            for c in range(NCH // 2):
                                for h in range(2):
                                    sl = slice(c * FC2 + h * FC,
                                               c * FC2 + (h + 1) * FC)
                                    nc.tensor.matmul(
                                        pss[c][:, h * FC:(h + 1) * FC],
                                        wsl(2 * i), prv[:, sl],
                                        start=False, stop=True)
                        for c in range(NCH // 2):
                            sl = slice(c * FC2, (c + 1) * FC2)
                            anchor = prox(prv[:, sl], pss[c][:], qt[:, sl])
                    cur, prv = prv, cur
                    if u_ == 0 and i == 4:
                        from concourse.tile import add_dep_helper
                        deferred = [
                            sy.dma_start(vw128[:], d_vw[0:N, :]),
                            nc.scalar.dma_start(vw16[:], d_vw[N:A2, :]),
                            sy.dma_start(afp[:], d_afp[:]),
                            nc.scalar.dma_start(pm[:], d_pm[:]),
                            sy.dma_start(afq128[:], d_afq[0:N, :]),
                            nc.scalar.dma_start(afq16[:], d_afq[N:A2, :]),
                            sy.dma_start(ysc[:], d_ysc[:]),
                            sy.dma_start(d_stga[:], ctb128[0:72, :]),
                            nc.scalar.dma_start(d_stgb[0:56, :],
                                                ctb128[72:N, :]),
                            nc.scalar.dma_start(d_stgb[56:72, :], ctb16[:]),
                        ]
                        for inst in deferred:
                            add_dep_helper(inst.ins, anchor.ins, sync=False,
                                           reason="defer off load ramp")

                # ---- pred^T = Af^T cf + pm, premult by vinv windows ----
                for c in range(NCH):
                    sl = slice(c * FC, (c + 1) * FC)
                    psp = psA.tile([N, FC], f32, tag="ps")
                    nc.tensor.matmul(psp[:], afp[:, 0:N], cur[:, sl],
                                     start=True, stop=False)
                    nc.tensor.matmul(psp[:], ones1[:, 0:N], pm[:, sl],
                                     start=False, stop=True)
                    nc.vector.tensor_mul(pp128[:, sl], psp[:], vw128[:, sl])
                    ps16 = psA.tile([16, FC], f32, tag="ps")
                    nc.tensor.matmul(ps16[:], afp[:, N:A2], cur[:, sl],
                                     start=True, stop=False)
                    nc.tensor.matmul(ps16[:], ones1[:, 0:16], pm[:, sl],
                                     start=False, stop=True)
                    nc.vector.tensor_mul(pp16[:, sl], ps16[:], vw16[:, sl])

                if u_ == 0:
                    # precompute next unroll's iter-0 matmuls (W @ cf) --
                    # runs in the otherwise PE-idle fold window
                    for c in range(NCH // 2):
                        ps = psA.tile([N, FC2], f32, tag="ps")
                        pre_ps.append(ps)
                        for h in range(2):
                            sl = slice(c * FC2 + h * FC,
                                       c * FC2 + (h + 1) * FC)
                            nc.tensor.matmul(ps[:, h * FC:(h + 1) * FC],
                                             wsl(0), cur[:, sl],
                                             start=True, stop=True)

                if u_ == 1:
                    # final unroll: ship premultiplied pred^T; the host
                    # does the (tiny) overlap-add fold in fp32
                    for c in range(NCH):
                        sl = slice(c * FC, (c + 1) * FC)
                        eng = (sy, nc.scalar)[c % 2]
                        eng.dma_start(d_pred[0:N, sl], pp128[:, sl])
                        eng.dma_start(d_pred[N:A2, sl], pp16[:, sl])
                    continue

                # ---- scatter-fold via DRAM staging: fused window
                # writes (DRAM dst: arbitrary strides legal), then
                # per-group loads back into the SBUF reduce tiles ----
                GRPS = [(0 + 12 * d, 12) for d in range(10)] + \
                       [(120, 8), (128, 4), (132, 12)]
                for gi, (k0, cnt) in enumerate(GRPS):
                    di, dj0 = divmod(k0, A)
                    if k0 < N:
                        s_ap = bass.AP(pp128[:].tensor, k0 * NP,
                                       [[NP, cnt], [1, NP]])
                    else:
                        s_ap = bass.AP(pp16[:].tensor, (k0 - N) * NP,
                                       [[NP, cnt], [1, NP]])
                    if k0 < 72:
                        d_ap = bass.AP(d_stga[:].tensor,
                                       k0 * PIX + di * HW + dj0,
                                       [[PIX + 1, cnt], [HW, PH], [1, PH]])
                    else:
                        d_ap = bass.AP(d_stgb[:].tensor,
                                       (k0 - 72) * PIX + di * HW + dj0,
                                       [[PIX + 1, cnt], [HW, PH], [1, PH]])
                    eng = (sy, nc.scalar)[gi % 2]
                    eng.dma_start(d_ap, s_ap)
                sy.dma_start(ctb128[0:72, :], d_stga[:])
                nc.scalar.dma_start(ctb128[72:N, :], d_stgb[0:56, :])
                nc.scalar.dma_start(ctb16[:], d_stgb[56:72, :])

                # ---- reduce + goal update ----
                for rc in range(NRC):
                    sl = slice(rc * RC, (rc + 1) * RC)
                    psr = psB.tile([1, RC], f32, tag="psr")
                    nc.tensor.matmul(psr[:], on128[:], ctb128[:, sl],
                                     start=True, stop=False)
                    nc.tensor.matmul(psr[:], on16[:], ctb16[:, sl],
                                     start=False, stop=True)
                    g = gst.tile([1, RC], bf16, tag="gb")
                    nc.vector.tensor_add(g[:], psr[:], ysc[:, sl])
                    sy.dma_start(d_goal[:, sl], g[:])

    nc.compile()
    return nc


_PROGRAM = None


def kernel(y, atoms, beta, mu):
    global _PROGRAM
    import concourse.mybir as mybir
    from concourse.bass_utils import run_bass_kernel_spmd

    y = np.asarray(y, np.float32)
    Af, wstack, mu_f, denom, vinv = _host_prep(
        np.asarray(atoms, np.float32), float(np.asarray(beta)),
        float(np.asarray(mu)))

    bfnp = mybir.dt.np(mybir.dt.bfloat16)
    afq = np.ascontiguousarray(mu_f * Af.T).astype(bfnp)     # [144,128]
    vw = np.ascontiguousarray(_im2col(vinv)).astype(bfnp)    # [144,4096]
    shared = {
        "wstack": wstack.astype(bfnp),
        "afq": afq,
        "afp": np.ascontiguousarray(Af).astype(bfnp),
        "vw": vw,
    }
    in_maps = []
    for b in range(B):
        img = y[b, 0]
        cols = _im2col(img)                                  # [144,4096]
        q0 = (mu_f * (Af @ cols)).astype(bfnp)               # [128,4096]
        pmv = cols.mean(axis=0, keepdims=True).astype(bfnp)  # [1,4096]
        ysc = (img / denom).reshape(1, PIX).astype(np.float32)
        in_maps.append({**shared, "q0": q0, "pmv": pmv, "ysc": ysc})

    if _PROGRAM is None:
        _PROGRAM = _build_program()
    res = run_bass_kernel_spmd(_PROGRAM, in_maps, list(range(B)))
    out = np.empty((B, 1, HW, HW), np.float32)
    for b in range(B):
        pred2 = np.asarray(res.results[b]["pred2"], np.float32)  # [144,4096]
        acc = in_maps[b]["ysc"].reshape(HW, HW).astype(np.float32).copy()
        pv = pred2.reshape(A2, PH, PH)
        for di in range(A):
            for dj in range(A):
                acc[di:di + PH, dj:dj + PH] += pv[di * A + dj]
        out[b, 0] = acc
    return out


if __name__ == "__main__":
    rng = np.random.default_rng(0)
    y = rng.standard_normal((B, 1, HW, HW), np.float32)
    atoms = rng.standard_normal((N, 1, A, A), np.float32) / 1500.0
    print(kernel(y, atoms, np.float32(0.1), np.float32(1.0)).shape)



# revision 3
# speedup vs baseline: 1.2375x; 1.2375x over previous
"""Trainium2 Bass kernel for nn_Dictionnary (convolutional sparse coding /
FISTA dictionary inference), data-parallel over the batch axis: each of the
8 NeuronCores processes one batch image independently (4096 patches/core).

Math (per unroll, mirrors the jax reference exactly):
  q' = mu * Af @ im2col(goal)                      [128, 4096]
  FISTA, 15 iters + 1 extra prox step, reformulated so the momentum is
  folded into pre-scaled weight matrices (W symmetric):
      s_i  = (1+b)W d_i + (-b)W d_{i-1} + q'       (2 matmuls, PSUM accum)
      d_i+1 = prox(s_i) = relu(s_i-lam) - relu(-s_i-lam)
  The iter-0 prox d0 = prox(q') is hosted; the goal image never
  materializes on device: goal_1 = G0 + vinv*fold(Af^T cf) with G0 and
  q_c1 = mu*Af@im2col(G0) precomputed on host, so the inter-unroll phase
  is fold -> ones-reduce -> im2col -> q-matmul (+ I @ q_c1 in PSUM).

Patch tensors that cross the image domain use a row-padded layout
[k, r*75+c] so the fold scatter and im2col gather DMAs move contiguous
2.4KB runs (the +1-elem diagonal stays on the DRAM-side outer dim).
All phases are chunked (1024-patch waves / 16-image-row groups) and
interleaved so the PE never idles long enough to drop its HAM clock.
"""
import numpy as np

N = 128          # atoms
A = 12           # atom size
A2 = 144         # atom pixels
B = 8            # batch
HW = 75
PH = 64          # patch grid
NP = PH * PH     # 4096 patches per core
PIX = HW * HW    # 5625
PIXP = PIX + 16  # padded plane (absorbs row-pad overrun)
PW = 75 * PH     # 4800: padded patch layout row stride * rows
LAM = 0.1
ITERS = 15
FC = 512         # free-dim chunk (one PSUM bank of fp32)
NCH = NP // FC   # 8 chunks
FC2 = 2 * FC     # superchunk
NSC = NP // FC2  # 4 superchunks
WV = 1024        # wave = 16 patch rows
NWV = NP // WV   # 4 waves

_PROX_OP = None


def _prox_np(u):
    return np.sign(u) * np.maximum(np.abs(u) - LAM, 0.0)


def _im2col(img):
    out = np.empty((A2, NP), np.float32)
    for di in range(A):
        for dj in range(A):
            out[di * A + dj] = img[di:di + PH, dj:dj + PH].reshape(-1)
    return out


def _fold(pl):
    # pl: [A2, PH, PH] -> [HW, HW] overlap-add
    acc = np.zeros((HW, HW), np.float32)
    for di in range(A):
        for dj in range(A):
            acc[di:di + PH, dj:dj + PH] += pl[di * A + dj]
    return acc


def _host_prep(atoms, beta, mu):
    beta = float(max(beta, 0.0))
    mu = float(max(mu, 0.0))
    Araw = atoms - atoms.mean(axis=(1, 2, 3), keepdims=True)
    Af = Araw.reshape(N, -1).astype(np.float64)
    Af = Af / np.linalg.norm(Af, axis=1, keepdims=True)
    Af = Af / (np.linalg.norm(Af, ord=2) * np.sqrt(mu))
    Af = Af.astype(np.float32)
    W = np.eye(N, dtype=np.float32) - np.float32(mu) * (Af @ Af.T)
    t = 1.0
    alphas = []
    for _ in range(ITERS):
        tn = (1.0 + np.sqrt(1.0 + 4.0 * t * t)) / 2.0
        alphas.append((t - 1.0) / tn)
        t = tn
    wstack = [W]
    for i in range(1, ITERS):
        b_ = np.float32(alphas[i - 1])
        wstack += [(1 + b_) * W, (-b_) * W]
    wstack = np.ascontiguousarray(np.stack(wstack))          # [29,128,128]
    div = np.zeros((HW, HW), np.float32)
    for di in range(A):
        for dj in range(A):
            div[di:di + PH, dj:dj + PH] += 1.0
    denom = 1.0 + beta * div
    vinv = (beta / denom).astype(np.float32)
    return Af, wstack, np.float32(mu), denom, vinv


def _get_prox_op():
    """Register (once) a fused DVE op: out = prox(in0 + in1, lam=imm2)."""
    global _PROX_OP
    if _PROX_OP is not None:
        return _PROX_OP
    import concourse.dve_ops as dve_ops
    from concourse.dve_spec import Spec, Src0, Src1, Zero, C2, relu, lower

    def _ref(in0, in1, s0, s1, imm2):
        u = in0.astype(np.float32) + in1.astype(np.float32)
        return np.maximum(u - imm2, 0.0) - np.maximum(-u - imm2, 0.0)

    spec = Spec(
        body=relu((Src0 + Src1) - C2) - relu((Zero - (Src0 + Src1)) - C2),
        reference=_ref,
    )
    op = dve_ops.DveOp("PROX_ADD_ANT", spec, subdim=False, uops_sha={})
    dve_ops.OPS.append(op)
    dve_ops.CUSTOM_DVE_SPECS[op.name] = op.spec
    dve_ops._SUB_OPCODE_FOR_NAME[op.name] = (
        dve_ops._CUSTOM_DVE_ROW_BASE + len(dve_ops.OPS) - 1)
    from concourse.dve_ops import DveOpSpec, has_src1, get_dve_sub_opcode
    for ver in ("v3", "v4"):
        res = DveOpSpec(name=op.name, opcode=get_dve_sub_opcode(op.name),
                        uops=lower(op.spec, ver=ver), rd1_en=has_src1(op.spec))
        op.uops_sha[ver] = res.sha(ver)
    _PROX_OP = op
    return op


# scatter/gather k-groups: consecutive dj within one di row, split at the
# 128-partition boundary -> (k0, cnt)
GRPS = [(12 * d, 12) for d in range(10)] + [(120, 8), (128, 4), (132, 12)]


def _build_program():
    import concourse.bacc as bacc
    import concourse.bass as bass
    import concourse.mybir as mybir
    import concourse.tile as tile
    from concourse.tile import add_dep_helper

    f32 = mybir.dt.float32
    bf16 = mybir.dt.bfloat16
    prox_op = _get_prox_op()

    nc = bacc.Bacc(None, target_bir_lowering=False, num_swdge_queues=4)

    d_wstack = nc.dram_tensor("wstack", [29, N, N], bf16, kind="ExternalInput")
    d_afq = nc.dram_tensor("afq", [A2, N], bf16, kind="ExternalInput")
    d_afp = nc.dram_tensor("afp", [N, A2], bf16, kind="ExternalInput")
    d_i128 = nc.dram_tensor("i128", [N, N], bf16, kind="ExternalInput")
    d_vw = nc.dram_tensor("vw", [A2, PW], bf16, kind="ExternalInput")
    d_q0 = nc.dram_tensor("q0", [N, NP], bf16, kind="ExternalInput")
    d_d0 = nc.dram_tensor("d0", [N, NP], bf16, kind="ExternalInput")
    d_qc1 = nc.dram_tensor("qc1", [N, NP], bf16, kind="ExternalInput")
    d_stg = nc.dram_tensor("stg", [A2, PIXP], bf16)
    d_goal = nc.dram_tensor("goalimg", [1, PIXP], bf16)
    d_pred = nc.dram_tensor("pred2", [A2, PW], bf16, kind="ExternalOutput")

    with tile.TileContext(nc) as tc:
        with (
            tc.tile_pool(name="cst", bufs=1) as cst,
            tc.tile_pool(name="psA", bufs=2, space="PSUM") as psA,
            tc.tile_pool(name="psB", bufs=4, space="PSUM") as psB,
        ):
            # ---- persistent tiles ----
            w_s = cst.tile([N, 29 * N], bf16)
            afq128 = cst.tile([N, N], bf16)
            afq16 = cst.tile([16, N], bf16)
            afp = cst.tile([N, A2], bf16)
            i128 = cst.tile([N, N], bf16)
            on128 = cst.tile([N, 1], bf16)
            on16 = cst.tile([16, 1], bf16)
            vw128 = cst.tile([N, PW], bf16)
            vw16 = cst.tile([16, PW], bf16)
            qt = cst.tile([N, NP], bf16)
            qc1 = cst.tile([N, NP], bf16)
            dA = cst.tile([N, NP], bf16)
            dB = cst.tile([N, NP], bf16)
            pp128 = cst.tile([N, PW], bf16)
            pp16 = cst.tile([16, PW], bf16)
            ctb128 = cst.tile([N, PIXP], bf16)
            ctb16 = cst.tile([16, PIXP], bf16)
            goal_sb = cst.tile([1, PIX], bf16)

            sy = nc.sync
            sc = nc.scalar

            def wsl(i):
                return w_s[:, i * N:(i + 1) * N]

            def prox(dst, ps_ap, q_ap):
                return nc.vector._custom_dve(prox_op, out=dst, in0=ps_ap,
                                             in1=q_ap, imm2=LAM)

            # ---- startup loads: only what iter 1..3 needs first ----
            # dA holds d0 = prox(q0) (hosted iter-0 prox); qt holds q0
            sy.dma_start(w_s[:, 1 * N:2 * N], d_wstack[1])
            for v in range(NWV):
                eng = (sc, sy)[v % 2]
                eng.dma_start(dA[:, v * WV:(v + 1) * WV],
                              d_d0[:, v * WV:(v + 1) * WV])
            for i in (3, 4):
                sy.dma_start(w_s[:, i * N:(i + 1) * N], d_wstack[i])
            for v in range(NWV):
                eng = (sy, sc)[v % 2]
                eng.dma_start(qt[:, v * WV:(v + 1) * WV],
                              d_q0[:, v * WV:(v + 1) * WV])
            for i in list(range(5, 29)) + [0, 2]:
                eng = (sy, sc)[i % 2]
                eng.dma_start(w_s[:, i * N:(i + 1) * N], d_wstack[i])
            nc.gpsimd.memset(on128[:], 1.0)
            nc.gpsimd.memset(on16[:], 1.0)
            nc.gpsimd.memset(pp128[:], 0.0)
            nc.gpsimd.memset(pp16[:], 0.0)
            nc.gpsimd.memset(ctb128[:], 0.0)
            nc.gpsimd.memset(ctb16[:], 0.0)

            cur, prv = dA, dB   # cur = c_i (starts at hosted d0), prv = c_{i-1}

            def fista_iter(w1, w2):
                """One FISTA step over all superchunks; returns last prox."""
                nonlocal cur, prv
                anchor = None
                for s in range(NSC):
                    ps = psA.tile([N, FC2], f32, tag="ps")
                    for h in range(2):
                        sl = slice(s * FC2 + h * FC, s * FC2 + (h + 1) * FC)
                        nc.tensor.matmul(ps[:, h * FC:(h + 1) * FC],
                                         w1, cur[:, sl],
                                         start=True, stop=w2 is None)
                        if w2 is not None:
                            nc.tensor.matmul(ps[:, h * FC:(h + 1) * FC],
                                             w2, prv[:, sl],
                                             start=False, stop=True)
                    sl2 = slice(s * FC2, (s + 1) * FC2)
                    anchor = prox(prv[:, sl2], ps[:], qt[:, sl2])
                cur, prv = prv, cur
                return anchor

            # ================= unroll 0: FISTA =================
            deferred_batches = {
                4: lambda: [
                    sy.dma_start(vw128[:], d_vw[0:N, :]),
                    sc.dma_start(vw16[:], d_vw[N:A2, :]),
                ],
                6: lambda: [
                    sy.dma_start(d_stg[0:N, :], ctb128[:]),
                    sc.dma_start(d_stg[N:A2, :], ctb16[:]),
                ],
                8: lambda: [
                    sy.dma_start(afp[:], d_afp[:]),
                    sc.dma_start(afq128[:], d_afq[0:N, :]),
                    sy.dma_start(afq16[:], d_afq[N:A2, :]),
                    sc.dma_start(i128[:], d_i128[:]),
                ],
                10: lambda: [
                    sy.dma_start(qc1[:, 0:NP // 2], d_qc1[:, 0:NP // 2]),
                    sc.dma_start(qc1[:, NP // 2:], d_qc1[:, NP // 2:]),
                ],
            }
            for i in range(1, ITERS + 1):
                if i == 1:
                    anchor = fista_iter(wsl(1), None)
                elif i == ITERS:
                    anchor = fista_iter(wsl(0), None)
                else:
                    anchor = fista_iter(wsl(2 * i - 1), wsl(2 * i))
                if i in deferred_batches:
                    for inst in deferred_batches[i]():
                        add_dep_helper(inst.ins, anchor.ins, sync=False,
                                       reason="defer off load ramp")

            # ============ pred + vinv-premult + fold scatter ============
            def pred_phase(final):
                dmas = 0
                for c in range(NCH):
                    sl = slice(c * FC, (c + 1) * FC)
                    # padded-layout dst AP: rows 8c..8c+8, 64 valid cols
                    po = c * 8 * 75
                    d128 = bass.AP(pp128[:].tensor, po,
                                   [[PW, N], [75, 8], [1, PH]])
                    d16 = bass.AP(pp16[:].tensor, po,
                                  [[PW, 16], [75, 8], [1, PH]])
                    v128 = bass.AP(vw128[:].tensor, po,
                                   [[PW, N], [75, 8], [1, PH]])
                    v16 = bass.AP(vw16[:].tensor, po,
                                  [[PW, 16], [75, 8], [1, PH]])
                    psp = psB.tile([N, FC], f32, tag="pb")
                    nc.tensor.matmul(psp[:], afp[:, 0:N], cur[:, sl],
                                     start=True, stop=True)
                    nc.vector.tensor_mul(d128, psp[:], v128)
                    ps16 = psB.tile([16, FC], f32, tag="pb")
                    nc.tensor.matmul(ps16[:], afp[:, N:A2], cur[:, sl],
                                     start=True, stop=True)
                    nc.vector.tensor_mul(d16, ps16[:], v16)
                    if final:
                        eng = (sy, sc)[c % 2]
                        eng.dma_start(
                            bass.AP(d_pred[:].tensor, po,
                                    [[PW, N], [75, 8], [1, PH]]), d128)
                        eng.dma_start(
                            bass.AP(d_pred[:].tensor, N * PW + po,
                                    [[PW, 16], [75, 8], [1, PH]]), d16)
                    elif c % 2 == 1:
                        # scatter wave w = chunks (c-1, c): contiguous
                        # 1200-elem runs into the padded staging planes
                        w = c // 2
                        for gi, (k0, cnt) in enumerate(GRPS):
                            di, dj0 = divmod(k0, A)
                            if k0 < N:
                                s_ap = bass.AP(pp128[:].tensor,
                                               k0 * PW + w * 1200,
                                               [[PW, cnt], [1, 1200]])
                            else:
                                s_ap = bass.AP(pp16[:].tensor,
                                               (k0 - N) * PW + w * 1200,
                                               [[PW, cnt], [1, 1200]])
                            d_ap = bass.AP(d_stg[:].tensor,
                                           k0 * PIXP + di * 75 + dj0 + w * 1200,
                                           [[PIXP + 1, cnt], [1, 1200]])
                            eng = (sy, sc)[(dmas := dmas + 1) % 2]
                            eng.dma_start(d_ap, s_ap)

            pred_phase(final=False)

            # ============ gather + reduce + goal rows ============
            # gather groups g: image rows [16g, 16g+16) (last: [64, 75))
            for g in range(5):
                r0 = 16 * g
                nr = 16 if g < 4 else 11
                sl = slice(r0 * 75, (r0 + nr) * 75)
                sy.dma_start(ctb128[0:N, sl], d_stg[0:N, sl])
                sc.dma_start(ctb16[:, sl], d_stg[N:A2, sl])
                # reduce in <=512-col chunks; 3 per group
                npx = nr * 75
                cw = 400 if g < 4 else 275
                for j in range(3):
                    rsl = slice(r0 * 75 + j * cw, r0 * 75 + (j + 1) * cw)
                    psr = psB.tile([1, cw], f32, tag="pb", name=f"psr{g}_{j}")
                    nc.tensor.matmul(psr[:], on128[:], ctb128[:, rsl],
                                     start=True, stop=False)
                    nc.tensor.matmul(psr[:], on16[:], ctb16[:, rsl],
                                     start=False, stop=True)
                    nc.scalar.copy(goal_sb[:, rsl], psr[:])
                eng = (sy, sc)[g % 2]
                eng.dma_start(d_goal[:, sl], goal_sb[:, sl])

            # ============ im2col gather + q rebuild + u1 iter 0 ============
            for v in range(NWV):
                # im2col wave v: patch rows [16v, 16v+16) from d_goal
                for gi, (k0, cnt) in enumerate(GRPS):
                    di, dj0 = divmod(k0, A)
                    s_ap = bass.AP(d_goal[:].tensor,
                                   di * 75 + dj0 + v * 1200,
                                   [[1, cnt], [1, 1200]])
                    if k0 < N:
                        d_ap = bass.AP(pp128[:].tensor, k0 * PW + v * 1200,
                                       [[PW, cnt], [1, 1200]])
                    else:
                        d_ap = bass.AP(pp16[:].tensor, (k0 - N) * PW + v * 1200,
                                       [[PW, cnt], [1, 1200]])
                    eng = (sy, sc)[gi % 2]
                    eng.dma_start(d_ap, s_ap)
                for h in range(2):
                    c = 2 * v + h
                    sl = slice(c * FC, (c + 1) * FC)
                    po = c * 8 * 75
                    r128 = bass.AP(pp128[:].tensor, po,
                                   [[PW, N], [75, 8], [1, PH]])
                    r16 = bass.AP(pp16[:].tensor, po,
                                  [[PW, 16], [75, 8], [1, PH]])
                    psq = psB.tile([N, FC], f32, tag="pb", name=f"psq{c}")
                    nc.tensor.matmul(psq[:], afq128[:], r128,
                                     start=True, stop=False)
                    nc.tensor.matmul(psq[:], afq16[:], r16,
                                     start=False, stop=False)
                    nc.tensor.matmul(psq[:], i128[:], qc1[:, sl],
                                     start=False, stop=True)
                    nc.scalar.copy(qt[:, sl], psq[:])

            # ================= unroll 1: FISTA =================
            for i in range(0, ITERS + 1):
                if i == 0 or i == ITERS:
                    fista_iter(wsl(0), None)
                else:
                    fista_iter(wsl(2 * i - 1), wsl(2 * i))

            # final pred, premultiplied by vinv windows; host does the fold
            pred_phase(final=True)

    nc.compile()
    return nc


_PROGRAM = None


def _make_in_maps(y, atoms, beta, mu):
    import concourse.mybir as mybir
    bfnp = mybir.dt.np(mybir.dt.bfloat16)
    y = np.asarray(y, np.float32)
    Af, wstack, mu_f, denom, vinv = _host_prep(
        np.asarray(atoms, np.float32), float(np.asarray(beta)),
        float(np.asarray(mu)))
    # padded vinv windows: vw_pad[k, r*75+c] = vinv[di+r, dj+c]
    vw_pad = np.zeros((A2, PW), np.float32)
    for di in range(A):
        for dj in range(A):
            k = di * A + dj
            vw_pad[k].reshape(PH, 75)[:, 0:PH] = \
                vinv[di:di + PH, dj:dj + PH]
    shared = {
        "wstack": wstack.astype(bfnp),
        "afq": np.ascontiguousarray(mu_f * Af.T).astype(bfnp),
        "afp": np.ascontiguousarray(Af).astype(bfnp),
        "i128": np.eye(N, dtype=np.float32).astype(bfnp),
        "vw": vw_pad.astype(bfnp),
    }
    in_maps = []
    g0s = []
    for b in range(y.shape[0]):
        img = y[b, 0]
        cols = _im2col(img)
        q0 = mu_f * (Af @ cols)
        d0 = _prox_np(q0)
        pm = cols.mean(axis=0)                       # [4096] patch means
        foldpm = _fold(np.broadcast_to(pm.reshape(1, PH, PH), (A2, PH, PH)))
        G0 = img / denom + vinv * foldpm
        qc1 = mu_f * (Af @ _im2col(G0))
        in_maps.append({**shared,
                        "q0": q0.astype(bfnp),
                        "d0": d0.astype(bfnp),
                        "qc1": qc1.astype(bfnp)})
        g0s.append(G0)
    return in_maps, g0s


def kernel(y, atoms, beta, mu):
    global _PROGRAM
    from concourse.bass_utils import run_bass_kernel_spmd

    in_maps, g0s = _make_in_maps(y, atoms, beta, mu)
    if _PROGRAM is None:
        _PROGRAM = _build_program()
    res = run_bass_kernel_spmd(_PROGRAM, in_maps, list(range(B)))
    out = np.empty((B, 1, HW, HW), np.float32)
    for b in range(B):
        pred2 = np.asarray(res.results[b]["pred2"], np.float32)  # [144,4800]
        pv = pred2.reshape(A2, PH, 75)[:, :, 0:PH]
        out[b, 0] = g0s[b] + _fold(pv)
    return out


if __name__ == "__main__":
    rng = np.random.default_rng(0)
    y = rng.standard_normal((B, 1, HW, HW), np.float32)
    atoms = rng.standard_normal((N, 1, A, A), np.float32) / 1500.0
    print(kernel(y, atoms, np.float32(0.1), np.float32(1.0)).shape)


# revision 11
# speedup vs baseline: 1.3729x; 1.1094x over previous
"""Trainium2 Bass kernel for nn_Dictionnary (convolutional sparse coding /
FISTA dictionary inference), data-parallel over the batch axis: each of the
8 NeuronCores processes one batch image independently (4096 patches/core).

Math (per unroll, mirrors the jax reference exactly):
  q' = mu * Af @ im2col(goal)                      [128, 4096]
  FISTA, 15 iters + 1 extra prox step, reformulated so the momentum is
  folded into pre-scaled weight matrices (W symmetric):
      s_i  = (1+b)W d_i + (-b)W d_{i-1} + q'       (2 matmuls, PSUM accum)
      d_i+1 = prox(s_i) = relu(s_i-lam) - relu(-s_i-lam)
  The iter-0 prox d0 = prox(q') is hosted; the goal image never
  materializes on device: goal_1 = G0 + vinv*fold(Af^T cf) with G0 and
  q_c1 = mu*Af@im2col(G0) precomputed on host, so the inter-unroll phase
  is fold -> ones-reduce -> im2col -> q-matmul (+ I @ q_c1 in PSUM).

Patch tensors that cross the image domain use a row-padded layout
[k, r*75+c] so the fold scatter and im2col gather DMAs move contiguous
2.4KB runs (the +1-elem diagonal stays on the DRAM-side outer dim).
All phases are chunked (1024-patch waves / 16-image-row groups) and
interleaved so the PE never idles long enough to drop its HAM clock.
"""
import numpy as np

N = 128          # atoms
A = 12           # atom size
A2 = 144         # atom pixels
B = 8            # batch
HW = 75
PH = 64          # patch grid
NP = PH * PH     # 4096 patches per core
PIX = HW * HW    # 5625
PIXP = PIX + 16  # padded plane (absorbs row-pad overrun)
PW = 75 * PH     # 4800: padded patch layout row stride * rows
LAM = 0.1
ITERS = 15
FC = 512         # free-dim chunk (one PSUM bank of fp32)
NCH = NP // FC   # 8 chunks
FC2 = 2 * FC     # superchunk
NSC = NP // FC2  # 4 superchunks
WV = 1024        # wave = 16 patch rows
NWV = NP // WV   # 4 waves

_PROX_OP = None


def _prox_np(u):
    return np.sign(u) * np.maximum(np.abs(u) - LAM, 0.0)


def _im2col(img):
    out = np.empty((A2, NP), np.float32)
    for di in range(A):
        for dj in range(A):
            out[di * A + dj] = img[di:di + PH, dj:dj + PH].reshape(-1)
    return out


def _fold(pl):
    # pl: [A2, PH, PH] -> [HW, HW] overlap-add
    acc = np.zeros((HW, HW), np.float32)
    for di in range(A):
        for dj in range(A):
            acc[di:di + PH, dj:dj + PH] += pl[di * A + dj]
    return acc


def _host_prep(atoms, beta, mu):
    beta = float(max(beta, 0.0))
    mu = float(max(mu, 0.0))
    Araw = atoms - atoms.mean(axis=(1, 2, 3), keepdims=True)
    Af = Araw.reshape(N, -1).astype(np.float64)
    Af = Af / np.linalg.norm(Af, axis=1, keepdims=True)
    Af = Af / (np.linalg.norm(Af, ord=2) * np.sqrt(mu))
    Af = Af.astype(np.float32)
    W = np.eye(N, dtype=np.float32) - np.float32(mu) * (Af @ Af.T)
    t = 1.0
    alphas = []
    for _ in range(ITERS):
        tn = (1.0 + np.sqrt(1.0 + 4.0 * t * t)) / 2.0
        alphas.append((t - 1.0) / tn)
        t = tn
    wstack = [W]
    for i in range(1, ITERS):
        b_ = np.float32(alphas[i - 1])
        wstack += [(1 + b_) * W, (-b_) * W]
    # reorder into first-use order so the device can load in 3 batched DMAs
    wstack = np.ascontiguousarray(
        np.stack([wstack[i] for i in WORDER]))               # [29,128,128]
    div = np.zeros((HW, HW), np.float32)
    for di in range(A):
        for dj in range(A):
            div[di:di + PH, dj:dj + PH] += 1.0
    denom = 1.0 + beta * div
    vinv = (beta / denom).astype(np.float32)
    return Af, wstack, np.float32(mu), denom, vinv


def _get_prox_op():
    """Register (once) a fused DVE op: out = prox(in0 + in1, lam=imm2)."""
    global _PROX_OP
    if _PROX_OP is not None:
        return _PROX_OP
    import concourse.dve_ops as dve_ops
    from concourse.dve_spec import Spec, Src0, Src1, Zero, C2, relu, lower

    def _ref(in0, in1, s0, s1, imm2):
        u = in0.astype(np.float32) + in1.astype(np.float32)
        return np.maximum(u - imm2, 0.0) - np.maximum(-u - imm2, 0.0)

    spec = Spec(
        body=relu((Src0 + Src1) - C2) - relu((Zero - (Src0 + Src1)) - C2),
        reference=_ref,
    )
    op = dve_ops.DveOp("PROX_ADD_ANT", spec, subdim=False, uops_sha={})
    dve_ops.OPS.append(op)
    dve_ops.CUSTOM_DVE_SPECS[op.name] = op.spec
    dve_ops._SUB_OPCODE_FOR_NAME[op.name] = (
        dve_ops._CUSTOM_DVE_ROW_BASE + len(dve_ops.OPS) - 1)
    from concourse.dve_ops import DveOpSpec, has_src1, get_dve_sub_opcode
    for ver in ("v3", "v4"):
        res = DveOpSpec(name=op.name, opcode=get_dve_sub_opcode(op.name),
                        uops=lower(op.spec, ver=ver), rd1_en=has_src1(op.spec))
        op.uops_sha[ver] = res.sha(ver)
    _PROX_OP = op
    return op


# weight-stack DRAM order = first-use order (w1; pairs; w0 & the unused w2)
WORDER = [1] + list(range(3, 29)) + [0, 2]
WPOS = {w: i for i, w in enumerate(WORDER)}

# scatter/gather k-groups merged over di via a 3-dim AP: (k0, ndi, ndj)
# covers planes k0 .. k0 + 12*ndi (ndj consecutive dj each), split at the
# 128-partition boundary of the pp tiles
GRP4 = [(0, 10, 12), (120, 1, 8), (128, 1, 4), (132, 1, 12)]
GRPS = [(12 * d, 12) for d in range(10)] + [(120, 8), (128, 4), (132, 12)]


def _build_program():
    import concourse.bacc as bacc
    import concourse.bass as bass
    import concourse.mybir as mybir
    import concourse.tile as tile
    from concourse.tile import add_dep_helper

    f32 = mybir.dt.float32
    bf16 = mybir.dt.bfloat16
    prox_op = _get_prox_op()

    nc = bacc.Bacc(None, target_bir_lowering=False, num_swdge_queues=4)

    d_wstack = nc.dram_tensor("wstack", [29, N, N], bf16, kind="ExternalInput")
    d_afq = nc.dram_tensor("afq", [A2, N], bf16, kind="ExternalInput")
    d_afp = nc.dram_tensor("afp", [N, A2], bf16, kind="ExternalInput")
    d_i128 = nc.dram_tensor("i128", [N, N], bf16, kind="ExternalInput")
    d_vw = nc.dram_tensor("vw", [A2, PW], bf16, kind="ExternalInput")
    d_q0 = nc.dram_tensor("q0", [N, NP], bf16, kind="ExternalInput")
    d_d0 = nc.dram_tensor("d0", [N, NP], bf16, kind="ExternalInput")
    d_qc1 = nc.dram_tensor("qc1", [N, NP], bf16, kind="ExternalInput")
    d_stg = nc.dram_tensor("stg", [A2, PIXP], bf16)
    d_goal = nc.dram_tensor("goalimg", [1, PIXP], bf16)
    d_pred = nc.dram_tensor("pred2", [A2, PW], bf16, kind="ExternalOutput")

    with tile.TileContext(nc) as tc:
        with (
            tc.tile_pool(name="cst", bufs=1) as cst,
            tc.tile_pool(name="psA", bufs=2, space="PSUM") as psA,
            tc.tile_pool(name="psB", bufs=4, space="PSUM") as psB,
        ):
            # ---- persistent tiles ----
            w_s = cst.tile([N, 29 * N], bf16)
            afq128 = cst.tile([N, N], bf16)
            afq16 = cst.tile([16, N], bf16)
            afp = cst.tile([N, A2], bf16)
            i128 = cst.tile([N, N], bf16)
            on128 = cst.tile([N, 1], bf16)
            on16 = cst.tile([16, 1], bf16)
            vw128 = cst.tile([N, PW], bf16)
            vw16 = cst.tile([16, PW], bf16)
            qt = cst.tile([N, NP], bf16)
            qc1 = cst.tile([N, NP], bf16)
            dA = cst.tile([N, NP], bf16)
            dB = cst.tile([N, NP], bf16)
            pp128 = cst.tile([N, PW], bf16)
            pp16 = cst.tile([16, PW], bf16)
            ctb128 = cst.tile([N, PIXP], bf16)
            ctb16 = cst.tile([16, PIXP], bf16)
            goal_sb = cst.tile([1, PIX], bf16)

            sy = nc.sync
            sc = nc.scalar

            def wsl(i):
                p = WPOS[i]
                return w_s[:, p * N:(p + 1) * N]

            def prox(dst, ps_ap, q_ap):
                return nc.vector._custom_dve(prox_op, out=dst, in0=ps_ap,
                                             in1=q_ap, imm2=LAM)

            def load_ws(a, b):
                # load wstack planes [a, b) (host order) into w_s cols
                src = bass.AP(d_wstack[:].tensor, a * N * N,
                              [[N, N], [N * N, b - a], [1, N]])
                dst = bass.AP(w_s[:].tensor, a * N,
                              [[29 * N, N], [N, b - a], [1, N]])
                return dst, src

            # ---- startup loads: only what iters 1..3 need first ----
            # dA holds d0 = prox(q0) (hosted iter-0 prox); qt holds q0
            sc.dma_start(dA[:, 0:WV], d_d0[:, 0:WV])
            for i in range(0, 3):
                sy.dma_start(w_s[:, i * N:(i + 1) * N], d_wstack[i])
            sc.dma_start(dA[:, WV:2 * WV], d_d0[:, WV:2 * WV])
            sy.dma_start(qt[:, 0:WV], d_q0[:, 0:WV])
            sc.dma_start(dA[:, 2 * WV:3 * WV], d_d0[:, 2 * WV:3 * WV])
            for i in range(3, 11):
                sy.dma_start(w_s[:, i * N:(i + 1) * N], d_wstack[i])
            sc.dma_start(dA[:, 3 * WV:4 * WV], d_d0[:, 3 * WV:4 * WV])
            sy.dma_start(qt[:, WV:2 * WV], d_q0[:, WV:2 * WV])
            sc.dma_start(qt[:, 2 * WV:3 * WV], d_q0[:, 2 * WV:3 * WV])
            for i in range(11, 29):
                sy.dma_start(w_s[:, i * N:(i + 1) * N], d_wstack[i])
            sc.dma_start(qt[:, 3 * WV:4 * WV], d_q0[:, 3 * WV:4 * WV])
            nc.gpsimd.memset(on128[:], 1.0)
            nc.gpsimd.memset(on16[:], 1.0)
            nc.gpsimd.memset(pp128[:], 0.0)
            nc.gpsimd.memset(pp16[:], 0.0)
            nc.gpsimd.memset(ctb128[:], 0.0)
            nc.gpsimd.memset(ctb16[:], 0.0)

            cur, prv = dA, dB   # cur = c_i (starts at hosted d0), prv = c_{i-1}

            def fista_iter(w1, w2):
                """One FISTA step over all superchunks; returns last prox."""
                nonlocal cur, prv
                anchor = None
                for s in range(NSC):
                    ps = psA.tile([N, FC2], f32, tag="ps")
                    for h in range(2):
                        sl = slice(s * FC2 + h * FC, s * FC2 + (h + 1) * FC)
                        nc.tensor.matmul(ps[:, h * FC:(h + 1) * FC],
                                         w1, cur[:, sl],
                                         start=True, stop=w2 is None)
                        if w2 is not None:
                            nc.tensor.matmul(ps[:, h * FC:(h + 1) * FC],
                                             w2, prv[:, sl],
                                             start=False, stop=True)
                    sl2 = slice(s * FC2, (s + 1) * FC2)
                    anchor = prox(prv[:, sl2], ps[:], qt[:, sl2])
                cur, prv = prv, cur
                return anchor

            # ================= unroll 0: FISTA =================
            deferred_batches = {
                4: lambda: [
                    sy.dma_start(vw128[:], d_vw[0:N, :]),
                    sc.dma_start(vw16[:], d_vw[N:A2, :]),
                ],
                6: lambda: [
                    sy.dma_start(d_stg[0:N, :], ctb128[:]),
                    sc.dma_start(d_stg[N:A2, :], ctb16[:]),
                ],
                8: lambda: [
                    sy.dma_start(afp[:], d_afp[:]),
                    sc.dma_start(afq128[:], d_afq[0:N, :]),
                    sy.dma_start(afq16[:], d_afq[N:A2, :]),
                    sc.dma_start(i128[:], d_i128[:]),
                ],
                10: lambda: [
                    sy.dma_start(qc1[:, 0:NP // 2], d_qc1[:, 0:NP // 2]),
                    sc.dma_start(qc1[:, NP // 2:], d_qc1[:, NP // 2:]),
                ],
            }
            for i in range(1, ITERS + 1):
                if i == 1:
                    anchor = fista_iter(wsl(1), None)
                elif i == ITERS:
                    anchor = fista_iter(wsl(0), None)
                else:
                    anchor = fista_iter(wsl(2 * i - 1), wsl(2 * i))
                if i in deferred_batches:
                    for inst in deferred_batches[i]():
                        add_dep_helper(inst.ins, anchor.ins, sync=False,
                                       reason="defer off load ramp")

            # ============ pred + vinv-premult + fold scatter ============
            def pred_phase(final):
                dmas = 0
                for c in range(NCH):
                    sl = slice(c * FC, (c + 1) * FC)
                    # padded-layout dst AP: rows 8c..8c+8, 64 valid cols
                    po = c * 8 * 75
                    d128 = bass.AP(pp128[:].tensor, po,
                                   [[PW, N], [75, 8], [1, PH]])
                    d16 = bass.AP(pp16[:].tensor, po,
                                  [[PW, 16], [75, 8], [1, PH]])
                    v128 = bass.AP(vw128[:].tensor, po,
                                   [[PW, N], [75, 8], [1, PH]])
                    v16 = bass.AP(vw16[:].tensor, po,
                                  [[PW, 16], [75, 8], [1, PH]])
                    psp = psB.tile([N, FC], f32, tag="pb")
                    nc.tensor.matmul(psp[:], afp[:, 0:N], cur[:, sl],
                                     start=True, stop=True)
                    nc.vector.tensor_mul(d128, psp[:], v128)
                    ps16 = psB.tile([16, FC], f32, tag="pb")
                    nc.tensor.matmul(ps16[:], afp[:, N:A2], cur[:, sl],
                                     start=True, stop=True)
                    nc.vector.tensor_mul(d16, ps16[:], v16)
                    if final and c % 4 == 3:
                        # ship padded pred in 2 half-image waves
                        w = c // 4
                        for t, r0, cnt in ((pp128, 0, N), (pp16, N, 16)):
                            s_ap = bass.AP(t[:].tensor, w * 2400,
                                           [[PW, cnt], [1, 2400]])
                            d_ap = bass.AP(d_pred[:].tensor,
                                           r0 * PW + w * 2400,
                                           [[PW, cnt], [1, 2400]])
                            eng = (sy, sc)[(dmas := dmas + 1) % 2]
                            eng.dma_start(d_ap, s_ap)
                    elif not final and c % 2 == 1:
                        # scatter wave w = chunks (c-1, c): contiguous
                        # 1200-elem runs into the padded staging planes;
                        # di is merged into the DRAM-side outer dim, the
                        # SBUF side stays a flat partition run
                        w = c // 2
                        for k0, ndi, ndj in GRP4:
                            di0, dj0 = divmod(k0, A)
                            t = pp128 if k0 < N else pp16
                            r0 = k0 if k0 < N else k0 - N
                            s_ap = bass.AP(t[:].tensor, r0 * PW + w * 1200,
                                           [[PW, ndi * ndj], [1, 1200]])
                            sdims = [[PIXP + 1, ndj], [1, 1200]]
                            if ndi > 1:
                                sdims = [[12 * PIXP + 75, ndi]] + sdims
                            d_ap = bass.AP(d_stg[:].tensor,
                                           k0 * PIXP + di0 * 75 + dj0
                                           + w * 1200, sdims)
                            eng = (sy, sc)[(dmas := dmas + 1) % 2]
                            eng.dma_start(d_ap, s_ap)

            pred_phase(final=False)

            # ============ gather + reduce + goal rows ============
            # gather groups g: image rows [16g, 16g+16) (last: [64, 75))
            GOAL_CHUNKS = {1: (0, 32), 3: (32, 64), 4: (64, 75)}
            for g in range(5):
                r0 = 16 * g
                nr = 16 if g < 4 else 11
                sl = slice(r0 * 75, (r0 + nr) * 75)
                sy.dma_start(ctb128[0:N, sl], d_stg[0:N, sl])
                sc.dma_start(ctb16[:, sl], d_stg[N:A2, sl])
                # reduce in <=512-col chunks; 3 per group
                cw = 400 if g < 4 else 275
                for j in range(3):
                    rsl = slice(r0 * 75 + j * cw, r0 * 75 + (j + 1) * cw)
                    psr = psB.tile([1, cw], f32, tag="pb", name=f"psr{g}_{j}")
                    nc.tensor.matmul(psr[:], on128[:], ctb128[:, rsl],
                                     start=True, stop=False)
                    nc.tensor.matmul(psr[:], on16[:], ctb16[:, rsl],
                                     start=False, stop=True)
                    nc.scalar.copy(goal_sb[:, rsl], psr[:])
                if g in GOAL_CHUNKS:
                    ra, rb = GOAL_CHUNKS[g]
                    gsl = slice(ra * 75, rb * 75)
                    eng = (sy, sc)[g % 2]
                    eng.dma_start(d_goal[:, gsl], goal_sb[:, gsl])

            # ============ im2col gather + q rebuild + u1 iter 0 ============
            for v in range(NWV):
                # im2col wave v: patch rows [16v, 16v+16) from d_goal
                for k0, ndi, ndj in GRP4:
                    di0, dj0 = divmod(k0, A)
                    t = pp128 if k0 < N else pp16
                    r0 = k0 if k0 < N else k0 - N
                    gdims = [[1, ndj], [1, 1200]]
                    if ndi > 1:
                        gdims = [[75, ndi]] + gdims
                    s_ap = bass.AP(d_goal[:].tensor,
                                   di0 * 75 + dj0 + v * 1200, gdims)
                    d_ap = bass.AP(t[:].tensor, r0 * PW + v * 1200,
                                   [[PW, ndi * ndj], [1, 1200]])
                    eng = (sy, sc)[(k0 + v) % 2]
                    eng.dma_start(d_ap, s_ap)
                for h in range(2):
                    c = 2 * v + h
                    sl = slice(c * FC, (c + 1) * FC)
                    po = c * 8 * 75
                    r128 = bass.AP(pp128[:].tensor, po,
                                   [[PW, N], [75, 8], [1, PH]])
                    r16 = bass.AP(pp16[:].tensor, po,
                                  [[PW, 16], [75, 8], [1, PH]])
                    psq = psB.tile([N, FC], f32, tag="pb", name=f"psq{c}")
                    nc.tensor.matmul(psq[:], afq128[:], r128,
                                     start=True, stop=False)
                    nc.tensor.matmul(psq[:], afq16[:], r16,
                                     start=False, stop=False)
                    nc.tensor.matmul(psq[:], i128[:], qc1[:, sl],
                                     start=False, stop=True)
                    nc.scalar.copy(qt[:, sl], psq[:])

            # ================= unroll 1: FISTA =================
            for i in range(0, ITERS + 1):
                if i == 0 or i == ITERS:
                    fista_iter(wsl(0), None)
                else:
                    fista_iter(wsl(2 * i - 1), wsl(2 * i))

            # final pred, premultiplied by vinv windows; host does the fold
            pred_phase(final=True)

    nc.compile()
    return nc


_PROGRAM = None


def _make_in_maps(y, atoms, beta, mu):
    import concourse.mybir as mybir
    bfnp = mybir.dt.np(mybir.dt.bfloat16)
    y = np.asarray(y, np.float32)
    Af, wstack, mu_f, denom, vinv = _host_prep(
        np.asarray(atoms, np.float32), float(np.asarray(beta)),
        float(np.asarray(mu)))
    # padded vinv windows: vw_pad[k, r*75+c] = vinv[di+r, dj+c]
    vw_pad = np.zeros((A2, PW), np.float32)
    for di in range(A):
        for dj in range(A):
            k = di * A + dj
            vw_pad[k].reshape(PH, 75)[:, 0:PH] = \
                vinv[di:di + PH, dj:dj + PH]
    shared = {
        "wstack": wstack.astype(bfnp),
        "afq": np.ascontiguousarray(mu_f * Af.T).astype(bfnp),
        "afp": np.ascontiguousarray(Af).astype(bfnp),
        "i128": np.eye(N, dtype=np.float32).astype(bfnp),
        "vw": vw_pad.astype(bfnp),
    }
    in_maps = []
    g0s = []
    for b in range(y.shape[0]):
        img = y[b, 0]
        cols = _im2col(img)
        q0 = mu_f * (Af @ cols)
        d0 = _prox_np(q0)
        pm = cols.mean(axis=0)                       # [4096] patch means
        foldpm = _fold(np.broadcast_to(pm.reshape(1, PH, PH), (A2, PH, PH)))
        G0 = img / denom + vinv * foldpm
        qc1 = mu_f * (Af @ _im2col(G0))
        in_maps.append({**shared,
                        "q0": q0.astype(bfnp),
                        "d0": d0.astype(bfnp),
                        "qc1": qc1.astype(bfnp)})
        g0s.append(G0)
    return in_maps, g0s


def kernel(y, atoms, beta, mu):
    global _PROGRAM
    from concourse.bass_utils import run_bass_kernel_spmd

    in_maps, g0s = _make_in_maps(y, atoms, beta, mu)
    if _PROGRAM is None:
        _PROGRAM = _build_program()
    res = run_bass_kernel_spmd(_PROGRAM, in_maps, list(range(B)))
    out = np.empty((B, 1, HW, HW), np.float32)
    for b in range(B):
        pred2 = np.asarray(res.results[b]["pred2"], np.float32)  # [144,4800]
        pv = pred2.reshape(A2, PH, 75)[:, :, 0:PH]
        out[b, 0] = g0s[b] + _fold(pv)
    return out


if __name__ == "__main__":
    rng = np.random.default_rng(0)
    y = rng.standard_normal((B, 1, HW, HW), np.float32)
    atoms = rng.standard_normal((N, 1, A, A), np.float32) / 1500.0
    print(kernel(y, atoms, np.float32(0.1), np.float32(1.0)).shape)


# revision 14
# speedup vs baseline: 1.4380x; 1.0474x over previous
"""Trainium2 Bass kernel for nn_Dictionnary (convolutional sparse coding /
FISTA dictionary inference), data-parallel over the batch axis: each of the
8 NeuronCores processes one batch image independently (4096 patches/core).

Math (per unroll, mirrors the jax reference exactly):
  q' = mu * Af @ im2col(goal)                      [128, 4096]
  FISTA, 15 iters + 1 extra prox step, reformulated so the momentum is
  folded into pre-scaled weight matrices (W symmetric):
      s_i  = (1+b)W d_i + (-b)W d_{i-1} + q'       (2 matmuls, PSUM accum)
      d_i+1 = prox(s_i) = relu(s_i-lam) - relu(-s_i-lam)
  The iter-0 prox d0 = prox(q') is hosted; the goal image never
  materializes on device: goal_1 = G0 + vinv*fold(Af^T cf) with G0 and
  q_c1 = mu*Af@im2col(G0) precomputed on host, so the inter-unroll phase
  is fold -> ones-reduce -> im2col -> q-matmul (+ I @ q_c1 in PSUM).

Patch tensors that cross the image domain use a row-padded layout
[k, r*75+c] so the fold scatter and im2col gather DMAs move contiguous
2.4KB runs (the +1-elem diagonal stays on the DRAM-side outer dim).
All phases are chunked (1024-patch waves / 16-image-row groups) and
interleaved so the PE never idles long enough to drop its HAM clock.
"""
import numpy as np

N = 128          # atoms
A = 12           # atom size
A2 = 144         # atom pixels
B = 8            # batch
HW = 75
PH = 64          # patch grid
NP = PH * PH     # 4096 patches per core
PIX = HW * HW    # 5625
PIXP = PIX + 16  # padded plane (absorbs row-pad overrun)
PW = 75 * PH     # 4800: padded patch layout row stride * rows
LAM = 0.1
ITERS = 15
FC = 512         # free-dim chunk (one PSUM bank of fp32)
NCH = NP // FC   # 8 chunks
FC2 = 2 * FC     # superchunk
NSC = NP // FC2  # 4 superchunks
WV = 1024        # wave = 16 patch rows
NWV = NP // WV   # 4 waves

_PROX_OP = None


def _prox_np(u):
    return np.sign(u) * np.maximum(np.abs(u) - LAM, 0.0)


def _im2col(img):
    out = np.empty((A2, NP), np.float32)
    for di in range(A):
        for dj in range(A):
            out[di * A + dj] = img[di:di + PH, dj:dj + PH].reshape(-1)
    return out


def _fold(pl):
    # pl: [A2, PH, PH] -> [HW, HW] overlap-add
    acc = np.zeros((HW, HW), np.float32)
    for di in range(A):
        for dj in range(A):
            acc[di:di + PH, dj:dj + PH] += pl[di * A + dj]
    return acc


def _host_prep(atoms, beta, mu):
    beta = float(max(beta, 0.0))
    mu = float(max(mu, 0.0))
    Araw = atoms - atoms.mean(axis=(1, 2, 3), keepdims=True)
    Af = Araw.reshape(N, -1).astype(np.float64)
    Af = Af / np.linalg.norm(Af, axis=1, keepdims=True)
    Af = Af / (np.linalg.norm(Af, ord=2) * np.sqrt(mu))
    Af = Af.astype(np.float32)
    W = np.eye(N, dtype=np.float32) - np.float32(mu) * (Af @ Af.T)
    t = 1.0
    alphas = []
    for _ in range(ITERS):
        tn = (1.0 + np.sqrt(1.0 + 4.0 * t * t)) / 2.0
        alphas.append((t - 1.0) / tn)
        t = tn
    wstack = [W]
    for i in range(1, ITERS):
        b_ = np.float32(alphas[i - 1])
        wstack += [(1 + b_) * W, (-b_) * W]
    # reorder into first-use order so the device can load in 3 batched DMAs
    wstack = np.ascontiguousarray(
        np.stack([wstack[i] for i in WORDER]))               # [29,128,128]
    div = np.zeros((HW, HW), np.float32)
    for di in range(A):
        for dj in range(A):
            div[di:di + PH, dj:dj + PH] += 1.0
    denom = 1.0 + beta * div
    vinv = (beta / denom).astype(np.float32)
    return Af, wstack, np.float32(mu), denom, vinv


def _get_prox_op():
    """Register (once) a fused DVE op: out = prox(in0 + in1, lam=imm2)."""
    global _PROX_OP
    if _PROX_OP is not None:
        return _PROX_OP
    import concourse.dve_ops as dve_ops
    from concourse.dve_spec import Spec, Src0, Src1, Zero, C2, relu, lower

    def _ref(in0, in1, s0, s1, imm2):
        u = in0.astype(np.float32) + in1.astype(np.float32)
        return np.maximum(u - imm2, 0.0) - np.maximum(-u - imm2, 0.0)

    spec = Spec(
        body=relu((Src0 + Src1) - C2) - relu((Zero - (Src0 + Src1)) - C2),
        reference=_ref,
    )
    op = dve_ops.DveOp("PROX_ADD_ANT", spec, subdim=False, uops_sha={})
    dve_ops.OPS.append(op)
    dve_ops.CUSTOM_DVE_SPECS[op.name] = op.spec
    dve_ops._SUB_OPCODE_FOR_NAME[op.name] = (
        dve_ops._CUSTOM_DVE_ROW_BASE + len(dve_ops.OPS) - 1)
    from concourse.dve_ops import DveOpSpec, has_src1, get_dve_sub_opcode
    for ver in ("v3", "v4"):
        res = DveOpSpec(name=op.name, opcode=get_dve_sub_opcode(op.name),
                        uops=lower(op.spec, ver=ver), rd1_en=has_src1(op.spec))
        op.uops_sha[ver] = res.sha(ver)
    _PROX_OP = op
    return op


# weight-stack DRAM order = first-use order (w1; pairs; w0 & the unused w2)
WORDER = [1] + list(range(3, 29)) + [0, 2]
WPOS = {w: i for i, w in enumerate(WORDER)}

# scatter/gather k-groups merged over di via a 3-dim AP: (k0, ndi, ndj)
# covers planes k0 .. k0 + 12*ndi (ndj consecutive dj each), split at the
# 128-partition boundary of the pp tiles
GRP4 = [(0, 10, 12), (120, 1, 8), (128, 1, 4), (132, 1, 12)]
GRPS = [(12 * d, 12) for d in range(10)] + [(120, 8), (128, 4), (132, 12)]


def _build_program():
    import concourse.bacc as bacc
    import concourse.bass as bass
    import concourse.mybir as mybir
    import concourse.tile as tile
    from concourse.tile import add_dep_helper

    f32 = mybir.dt.float32
    bf16 = mybir.dt.bfloat16
    prox_op = _get_prox_op()

    nc = bacc.Bacc(None, target_bir_lowering=False, num_swdge_queues=4)

    d_wstack = nc.dram_tensor("wstack", [29, N, N], bf16, kind="ExternalInput")
    d_afq = nc.dram_tensor("afq", [A2, N], bf16, kind="ExternalInput")
    d_afp = nc.dram_tensor("afp", [N, A2], bf16, kind="ExternalInput")
    d_i128 = nc.dram_tensor("i128", [N, N], bf16, kind="ExternalInput")
    d_vw = nc.dram_tensor("vw", [A2, PW], bf16, kind="ExternalInput")
    d_q0 = nc.dram_tensor("q0", [N, NP], bf16, kind="ExternalInput")
    d_d0 = nc.dram_tensor("d0", [N, NP], bf16, kind="ExternalInput")
    d_qc1 = nc.dram_tensor("qc1", [N, NP], bf16, kind="ExternalInput")
    d_stg = nc.dram_tensor("stg", [A2, PIXP], bf16)
    d_goal = nc.dram_tensor("goalimg", [1, PIXP], bf16)
    d_pred = nc.dram_tensor("pred2", [A2, PW], bf16, kind="ExternalOutput")

    with tile.TileContext(nc) as tc:
        with (
            tc.tile_pool(name="cst", bufs=1) as cst,
            tc.tile_pool(name="psA", bufs=2, space="PSUM") as psA,
            tc.tile_pool(name="psB", bufs=4, space="PSUM") as psB,
        ):
            # ---- persistent tiles ----
            w_s = cst.tile([N, 29 * N], bf16)
            afq128 = cst.tile([N, N], bf16)
            afq16 = cst.tile([16, N], bf16)
            afp = cst.tile([N, A2], bf16)
            i128 = cst.tile([N, N], bf16)
            on128 = cst.tile([N, 1], bf16)
            on16 = cst.tile([16, 1], bf16)
            vw128 = cst.tile([N, PW], bf16)
            vw16 = cst.tile([16, PW], bf16)
            qt = cst.tile([N, NP], bf16)
            qc1 = cst.tile([N, NP], bf16)
            dA = cst.tile([N, NP], bf16)
            dB = cst.tile([N, NP], bf16)
            pp128 = cst.tile([N, PW], bf16)
            pp16 = cst.tile([16, PW], bf16)
            ctb128 = cst.tile([N, PIXP], bf16)
            ctb16 = cst.tile([16, PIXP], bf16)
            goal_sb = cst.tile([1, PIX], bf16)

            sy = nc.sync
            sc = nc.scalar

            def wsl(i):
                p = WPOS[i]
                return w_s[:, p * N:(p + 1) * N]

            def prox(dst, ps_ap, q_ap):
                return nc.vector._custom_dve(prox_op, out=dst, in0=ps_ap,
                                             in1=q_ap, imm2=LAM)

            def load_ws(a, b):
                # load wstack planes [a, b) (host order) into w_s cols
                src = bass.AP(d_wstack[:].tensor, a * N * N,
                              [[N, N], [N * N, b - a], [1, N]])
                dst = bass.AP(w_s[:].tensor, a * N,
                              [[29 * N, N], [N, b - a], [1, N]])
                return dst, src

            # ---- startup loads: only what iters 1..3 need first ----
            # dA holds d0 = prox(q0) (hosted iter-0 prox); qt holds q0
            sc.dma_start(dA[:, 0:WV], d_d0[:, 0:WV])
            for i in range(0, 3):
                sy.dma_start(w_s[:, i * N:(i + 1) * N], d_wstack[i])
            sc.dma_start(dA[:, WV:2 * WV], d_d0[:, WV:2 * WV])
            sy.dma_start(qt[:, 0:WV], d_q0[:, 0:WV])
            sc.dma_start(dA[:, 2 * WV:3 * WV], d_d0[:, 2 * WV:3 * WV])
            for i in range(3, 11):
                sy.dma_start(w_s[:, i * N:(i + 1) * N], d_wstack[i])
            sc.dma_start(dA[:, 3 * WV:4 * WV], d_d0[:, 3 * WV:4 * WV])
            sy.dma_start(qt[:, WV:2 * WV], d_q0[:, WV:2 * WV])
            sc.dma_start(qt[:, 2 * WV:3 * WV], d_q0[:, 2 * WV:3 * WV])
            for i in range(11, 29):
                sy.dma_start(w_s[:, i * N:(i + 1) * N], d_wstack[i])
            sc.dma_start(qt[:, 3 * WV:4 * WV], d_q0[:, 3 * WV:4 * WV])
            nc.gpsimd.memset(on128[:], 1.0)
            nc.gpsimd.memset(on16[:], 1.0)
            nc.gpsimd.memset(pp128[:], 0.0)
            nc.gpsimd.memset(pp16[:], 0.0)
            nc.gpsimd.memset(ctb128[:], 0.0)
            nc.gpsimd.memset(ctb16[:], 0.0)

            cur, prv = dA, dB   # cur = c_i (starts at hosted d0), prv = c_{i-1}

            def fista_step(s, w1, w2):
                """One FISTA superchunk: matmul(s) + fused prox."""
                ps = psA.tile([N, FC2], f32, tag="ps")
                for h in range(2):
                    sl = slice(s * FC2 + h * FC, s * FC2 + (h + 1) * FC)
                    nc.tensor.matmul(ps[:, h * FC:(h + 1) * FC],
                                     w1, cur[:, sl],
                                     start=True, stop=w2 is None)
                    if w2 is not None:
                        nc.tensor.matmul(ps[:, h * FC:(h + 1) * FC],
                                         w2, prv[:, sl],
                                         start=False, stop=True)
                sl2 = slice(s * FC2, (s + 1) * FC2)
                return prox(prv[:, sl2], ps[:], qt[:, sl2])

            def fista_iter(w1, w2):
                nonlocal cur, prv
                anchor = None
                for s in range(NSC):
                    anchor = fista_step(s, w1, w2)
                cur, prv = prv, cur
                return anchor

            # mid-kernel loads go on the idle gpsimd SWDGE queues so they
            # never contend with the HWDGE (sync/scalar) critical streams
            gp = nc.gpsimd
            gp.dma_start(d_stg[0:N, :], ctb128[:])
            gp.dma_start(d_stg[N:A2, :], ctb16[:])
            gp.dma_start(vw128[:], d_vw[0:N, :])
            gp.dma_start(vw16[:], d_vw[N:A2, :])
            gp.dma_start(afp[:], d_afp[:])
            gp.dma_start(afq128[:], d_afq[0:N, :])
            gp.dma_start(afq16[:], d_afq[N:A2, :])
            gp.dma_start(i128[:], d_i128[:])
            gp.dma_start(qc1[:, 0:NP // 2], d_qc1[:, 0:NP // 2])
            gp.dma_start(qc1[:, NP // 2:], d_qc1[:, NP // 2:])

            # ================= unroll 0: FISTA =================
            for i in range(1, ITERS + 1):
                if i == 1:
                    fista_iter(wsl(1), None)
                elif i == ITERS:
                    fista_iter(wsl(0), None)
                else:
                    fista_iter(wsl(2 * i - 1), wsl(2 * i))

            # ============ pred + vinv-premult + fold scatter ============
            def pred_phase(final):
                dmas = 0
                for c in range(NCH):
                    sl = slice(c * FC, (c + 1) * FC)
                    # padded-layout dst AP: rows 8c..8c+8, 64 valid cols
                    po = c * 8 * 75
                    d128 = bass.AP(pp128[:].tensor, po,
                                   [[PW, N], [75, 8], [1, PH]])
                    d16 = bass.AP(pp16[:].tensor, po,
                                  [[PW, 16], [75, 8], [1, PH]])
                    psp = psB.tile([N, FC], f32, tag="pb")
                    nc.tensor.matmul(psp[:], afp[:, 0:N], cur[:, sl],
                                     start=True, stop=True)
                    ps16 = psB.tile([16, FC], f32, tag="pb")
                    nc.tensor.matmul(ps16[:], afp[:, N:A2], cur[:, sl],
                                     start=True, stop=True)
                    if final:
                        # raw pred out; host applies vinv inside its fold
                        (nc.scalar.copy if c % 2 else nc.vector.tensor_copy)(
                            d128, psp[:])
                        (nc.vector.tensor_copy if c % 2 else nc.scalar.copy)(
                            d16, ps16[:])
                    else:
                        v128 = bass.AP(vw128[:].tensor, po,
                                       [[PW, N], [75, 8], [1, PH]])
                        v16 = bass.AP(vw16[:].tensor, po,
                                      [[PW, 16], [75, 8], [1, PH]])
                        nc.vector.tensor_mul(d128, psp[:], v128)
                        nc.vector.tensor_mul(d16, ps16[:], v16)
                    if final and c % 4 == 3:
                        # ship padded pred in 2 half-image waves
                        w = c // 4
                        for t, r0, cnt in ((pp128, 0, N), (pp16, N, 16)):
                            s_ap = bass.AP(t[:].tensor, w * 2400,
                                           [[PW, cnt], [1, 2400]])
                            d_ap = bass.AP(d_pred[:].tensor,
                                           r0 * PW + w * 2400,
                                           [[PW, cnt], [1, 2400]])
                            eng = (sy, sc)[(dmas := dmas + 1) % 2]
                            eng.dma_start(d_ap, s_ap)
                    elif not final and c % 2 == 1:
                        # scatter wave w = chunks (c-1, c): contiguous
                        # 1200-elem runs into the padded staging planes;
                        # di is merged into the DRAM-side outer dim, the
                        # SBUF side stays a flat partition run
                        w = c // 2
                        for k0, ndi, ndj in GRP4:
                            di0, dj0 = divmod(k0, A)
                            t = pp128 if k0 < N else pp16
                            r0 = k0 if k0 < N else k0 - N
                            s_ap = bass.AP(t[:].tensor, r0 * PW + w * 1200,
                                           [[PW, ndi * ndj], [1, 1200]])
                            sdims = [[PIXP + 1, ndj], [1, 1200]]
                            if ndi > 1:
                                sdims = [[12 * PIXP + 75, ndi]] + sdims
                            d_ap = bass.AP(d_stg[:].tensor,
                                           k0 * PIXP + di0 * 75 + dj0
                                           + w * 1200, sdims)
                            eng = (sy, sc)[(dmas := dmas + 1) % 2]
                            eng.dma_start(d_ap, s_ap)

            pred_phase(final=False)

            # ============ gather + reduce + goal rows ============
            # gather groups g: image rows [16g, 16g+16) (last: [64, 75))
            GOAL_CHUNKS = {1: (0, 32), 3: (32, 64), 4: (64, 75)}
            for g in range(5):
                r0 = 16 * g
                nr = 16 if g < 4 else 11
                sl = slice(r0 * 75, (r0 + nr) * 75)
                sy.dma_start(ctb128[0:N, sl], d_stg[0:N, sl])
                sc.dma_start(ctb16[:, sl], d_stg[N:A2, sl])
                # reduce in <=512-col chunks; 3 per group
                cw = 400 if g < 4 else 275
                for j in range(3):
                    rsl = slice(r0 * 75 + j * cw, r0 * 75 + (j + 1) * cw)
                    psr = psB.tile([1, cw], f32, tag="pb", name=f"psr{g}_{j}")
                    nc.tensor.matmul(psr[:], on128[:], ctb128[:, rsl],
                                     start=True, stop=False)
                    nc.tensor.matmul(psr[:], on16[:], ctb16[:, rsl],
                                     start=False, stop=True)
                    nc.scalar.copy(goal_sb[:, rsl], psr[:])
                if g in GOAL_CHUNKS:
                    ra, rb = GOAL_CHUNKS[g]
                    gsl = slice(ra * 75, rb * 75)
                    eng = (sy, sc)[g % 2]
                    eng.dma_start(d_goal[:, gsl], goal_sb[:, gsl])

            # ============ im2col gather + q rebuild + u1 iter 0 ============
            for v in range(NWV):
                # im2col wave v: patch rows [16v, 16v+16) from d_goal
                for k0, ndi, ndj in GRP4:
                    di0, dj0 = divmod(k0, A)
                    t = pp128 if k0 < N else pp16
                    r0 = k0 if k0 < N else k0 - N
                    gdims = [[1, ndj], [1, 1200]]
                    if ndi > 1:
                        gdims = [[75, ndi]] + gdims
                    s_ap = bass.AP(d_goal[:].tensor,
                                   di0 * 75 + dj0 + v * 1200, gdims)
                    d_ap = bass.AP(t[:].tensor, r0 * PW + v * 1200,
                                   [[PW, ndi * ndj], [1, 1200]])
                    eng = (sy, sc)[(k0 + v) % 2]
                    eng.dma_start(d_ap, s_ap)
                for h in range(2):
                    c = 2 * v + h
                    sl = slice(c * FC, (c + 1) * FC)
                    po = c * 8 * 75
                    r128 = bass.AP(pp128[:].tensor, po,
                                   [[PW, N], [75, 8], [1, PH]])
                    r16 = bass.AP(pp16[:].tensor, po,
                                  [[PW, 16], [75, 8], [1, PH]])
                    psq = psB.tile([N, FC], f32, tag="pb", name=f"psq{c}")
                    nc.tensor.matmul(psq[:], afq128[:], r128,
                                     start=True, stop=False)
                    nc.tensor.matmul(psq[:], afq16[:], r16,
                                     start=False, stop=False)
                    nc.tensor.matmul(psq[:], i128[:], qc1[:, sl],
                                     start=False, stop=True)
                    nc.scalar.copy(qt[:, sl], psq[:])
                # u1 iter-0 for superchunk v follows its own q wave so the
                # PE FIFO isn't blocked behind later waves' im2col
                fista_step(v, wsl(0), None)
            cur, prv = prv, cur

            # ================= unroll 1: FISTA =================
            for i in range(1, ITERS + 1):
                if i == ITERS:
                    fista_iter(wsl(0), None)
                else:
                    fista_iter(wsl(2 * i - 1), wsl(2 * i))

            # final pred, premultiplied by vinv windows; host does the fold
            pred_phase(final=True)

    nc.compile()
    return nc


_PROGRAM = None


def _make_in_maps(y, atoms, beta, mu):
    import concourse.mybir as mybir
    bfnp = mybir.dt.np(mybir.dt.bfloat16)
    y = np.asarray(y, np.float32)
    Af, wstack, mu_f, denom, vinv = _host_prep(
        np.asarray(atoms, np.float32), float(np.asarray(beta)),
        float(np.asarray(mu)))
    # padded vinv windows: vw_pad[k, r*75+c] = vinv[di+r, dj+c]
    vw_pad = np.zeros((A2, PW), np.float32)
    for di in range(A):
        for dj in range(A):
            k = di * A + dj
            vw_pad[k].reshape(PH, 75)[:, 0:PH] = \
                vinv[di:di + PH, dj:dj + PH]
    shared = {
        "wstack": wstack.astype(bfnp),
        "afq": np.ascontiguousarray(mu_f * Af.T).astype(bfnp),
        "afp": np.ascontiguousarray(Af).astype(bfnp),
        "i128": np.eye(N, dtype=np.float32).astype(bfnp),
        "vw": vw_pad.astype(bfnp),
    }
    in_maps = []
    g0s = []
    vinvs = []
    for b in range(y.shape[0]):
        img = y[b, 0]
        cols = _im2col(img)
        q0 = mu_f * (Af @ cols)
        d0 = _prox_np(q0)
        pm = cols.mean(axis=0)                       # [4096] patch means
        foldpm = _fold(np.broadcast_to(pm.reshape(1, PH, PH), (A2, PH, PH)))
        G0 = img / denom + vinv * foldpm
        qc1 = mu_f * (Af @ _im2col(G0))
        in_maps.append({**shared,
                        "q0": q0.astype(bfnp),
                        "d0": d0.astype(bfnp),
                        "qc1": qc1.astype(bfnp)})
        g0s.append(G0)
        vinvs.append(vinv)
    return in_maps, g0s, vinvs


def kernel(y, atoms, beta, mu):
    global _PROGRAM
    from concourse.bass_utils import run_bass_kernel_spmd

    in_maps, g0s, vinvs = _make_in_maps(y, atoms, beta, mu)
    if _PROGRAM is None:
        _PROGRAM = _build_program()
    res = run_bass_kernel_spmd(_PROGRAM, in_maps, list(range(B)))
    out = np.empty((B, 1, HW, HW), np.float32)
    for b in range(B):
        pred2 = np.asarray(res.results[b]["pred2"], np.float32)  # [144,4800]
        pv = pred2.reshape(A2, PH, 75)[:, :, 0:PH]
        out[b, 0] = g0s[b] + vinvs[b] * _fold(pv)
    return out


if __name__ == "__main__":
    rng = np.random.default_rng(0)
    y = rng.standard_normal((B, 1, HW, HW), np.float32)
    atoms = rng.standard_normal((N, 1, A, A), np.float32) / 1500.0
    print(kernel(y, atoms, np.float32(0.1), np.float32(1.0)).shape)
